# revision 8
# baseline (speedup 1.0000x reference)
"""Trainium2 Bass kernel for nn_CrossAttention (B=2, S=2048, E=1024, H=16, ctx=768).

Sharding: 4-way tensor-parallel over heads x 2-way data-parallel over batch.
Core c handles batch c//4 and heads 4*(c%4) .. 4*(c%4)+3.

Software-pipelined single-pass design:
  The attention inner loop is ACT(exp)-bound (~1.04 us per [128,1024] exp
  tile) while its PE work (scores pair + av pair) is only ~0.76 us, so all
  projection / output-projection matmuls are emitted as *fillers* inside
  the attention windows to run in the PE slack.  Windows are p-major
  ((sc,p)=(0..3,0) then (0..3,1)) so the p=1 projections have 3+ windows
  of slack to complete, and out-proj of chunk sc runs inside window
  (sc+1, 1).

Per-core dataflow (fp16 operands, fp32 PSUM):
  qT/kT = W-stationary projections producing [dh, S] layouts (bias added
          during the DVE PSUM->SBUF drain)
  v2    = ctxT-tile-stationary projection into [S-tile, head, v|ones]
          (bias added via a host-replicated [128,256] table on DVE)
  scT   = kT-tile x qT (K=64); head pair emitted on PE row groups 0/64
  exp   = ScalarE, fused 1/sqrt(dh) scale, PSUM -> SBUF fp16, one
          [128,1024] tile per (t, head-pair); 1-deep scores lookahead
  av/Z  = [v_h | ones] stationary: rows 0:64 unnormalized out.T, rows
          64:128 softmax denominator, pair packed in one [128,1024] PSUM
  norm  = single DVE divide per head (numerator rows / denominator rows)
  out   = avT-stationary x Wo, fp16 partial [S, E] per core

Host side: pre-transpose x/context, slice weights per head group, fp16
cast; sum the 4 per-batch fp16 partials + bo in fp32 on host.
"""
import numpy as np

import concourse.bass as bass
import concourse.mybir as mybir
import concourse.tile as tile
from concourse import bacc, bass_utils

F16 = mybir.dt.float16
F32 = mybir.dt.float32
AF = mybir.ActivationFunctionType
OP = mybir.AluOpType

B, S, E, C, H, DH = 2, 2048, 1024, 768, 16, 64
N_CORES = 8
GROUPS = 4            # head groups (tensor parallel)
HPG = H // GROUPS     # heads per group = 4
DSL = HPG * DH        # feature slice per core = 256
KT_E = E // 128       # 8 k-tiles for x projections
KT_C = C // 128       # 6 k-tiles for context projections
SCK = S // 512        # 4 s-chunks
TT = S // 128         # 16 t-tiles

_NC_CACHE = {}


def _build_nc():
    nc = bacc.Bacc("TRN2", target_bir_lowering=False, debug=False,
                   num_devices=N_CORES)

    xT = nc.dram_tensor("xT", [E, S], F16, kind="ExternalInput").ap()
    ctxT = nc.dram_tensor("ctxT", [C, S], F16, kind="ExternalInput").ap()
    wq = nc.dram_tensor("wq", [E, DSL], F16, kind="ExternalInput").ap()
    wk = nc.dram_tensor("wk", [C, DSL], F16, kind="ExternalInput").ap()
    wv = nc.dram_tensor("wv", [C, DSL], F16, kind="ExternalInput").ap()
    wo = nc.dram_tensor("wo", [DSL, E], F16, kind="ExternalInput").ap()
    bq = nc.dram_tensor("bq", [128, 2], F32, kind="ExternalInput").ap()
    bk = nc.dram_tensor("bk", [128, 2], F32, kind="ExternalInput").ap()
    bvr = nc.dram_tensor("bvr", [128, DSL], F32, kind="ExternalInput").ap()
    out = nc.dram_tensor("out", [S, E], F16, kind="ExternalOutput").ap()

    xT_r = xT.rearrange("(o p) s -> p o s", p=128)
    ctxT_r = ctxT.rearrange("(o p) s -> p o s", p=128)
    wq_r = wq.rearrange("(o p) m -> p o m", p=128)
    wk_r = wk.rearrange("(o p) m -> p o m", p=128)

    with tile.TileContext(nc) as tc:
        with (
            tc.tile_pool(name="const", bufs=1) as cpool,
            tc.tile_pool(name="ex", bufs=4) as expool,
            tc.tile_pool(name="os", bufs=4) as ospool,
            tc.tile_pool(name="rz", bufs=1) as rzpool,
        ):
            wq_sb = cpool.tile([128, KT_E, DSL], F16)
            wk_sb = cpool.tile([128, KT_C, DSL], F16)
            wv_sb = cpool.tile([128, KT_C, DSL], F16)
            wo_sb = cpool.tile([128, 2, E], F16)
            bq_sb = cpool.tile([128, 2], F32)
            bk_sb = cpool.tile([128, 2], F32)
            bvr_sb = cpool.tile([128, DSL], F32)
            warm_sb = cpool.tile([1, 8], F32)
            ctxT_sb = cpool.tile([128, KT_C, S], F16)
            xT_sb = cpool.tile([128, KT_E, S], F16)

            qT_sb = cpool.tile([128, 2, S], F16)
            kT_sb = cpool.tile([128, 2, S], F16)
            # per (t, head): 128 cols = [v_h (64) | ones (64)] so one matmul
            # yields av rows 0:64 and the replicated softmax denominator
            # rows 64:128 in a single PSUM bank
            v2_sb = cpool.tile([128, TT, HPG, 128], F16)
            avT_sb = cpool.tile([128, 2, S], F16)

            # ---- DMAs in need-order, few large issues: each dma_start
            # costs ~625ns of serial SP-sequencer issue time, while one
            # issue's descriptors spread across all 16 DMA queues ----
            nc.sync.dma_start(wk_sb[:], wk_r[:])
            nc.sync.dma_start(ctxT_sb[:, 0:3, 0:512], ctxT_r[:, 0:3, 0:512])
            nc.sync.dma_start(ctxT_sb[:, 3:6, 0:512], ctxT_r[:, 3:6, 0:512])
            nc.sync.dma_start(wq_sb[:], wq_r[:])
            nc.sync.dma_start(xT_sb[:, 0:4, 0:512], xT_r[:, 0:4, 0:512])
            nc.sync.dma_start(xT_sb[:, 4:8, 0:512], xT_r[:, 4:8, 0:512])
            nc.sync.dma_start(bk_sb[:], bk[:])
            nc.sync.dma_start(bq_sb[:], bq[:])
            nc.sync.dma_start(wv_sb[:], wv.rearrange("(o p) m -> p o m", p=128))
            nc.sync.dma_start(bvr_sb[:], bvr[:])
            nc.sync.dma_start(ctxT_sb[:, :, 512:2048], ctxT_r[:, :, 512:2048])
            nc.sync.dma_start(xT_sb[:, :, 512:2048], xT_r[:, :, 512:2048])
            nc.sync.dma_start(wo_sb[:], wo.rearrange("(l p) n -> p l n", p=128))

            # ones halves of v2 (written once; v halves overwritten by proj)
            nc.vector.memset(v2_sb[:, :, :, DH:128], 1.0)
            nc.vector.memset(warm_sb[:], 0.0)
            # pull the exp table load off the critical path
            nc.scalar.activation(warm_sb[:], warm_sb[:], AF.Exp)

            with (
                tc.tile_pool(name="psc", bufs=2, space="PSUM") as psc,
                tc.tile_pool(name="pproj", bufs=2, space="PSUM") as pproj,
                tc.tile_pool(name="pavz", bufs=1, space="PSUM") as pavz,
            ):
                uid = [0]

                def qk_unit(dst, w_sb, b_sb, src, nk, p, ch, half):
                    """Half of one 512-col projection chunk (contract split).

                    half=0 allocates the psum + first nk//2 matmuls;
                    half=1 finishes the contraction + DVE bias drain.
                    Returns state via closure dict."""
                    pass  # replaced below

                # projection chunk as two emission units sharing one psum
                def make_qk_units(dst, w_sb, b_sb, src, nk, p, ch):
                    st = {}
                    csl = slice(ch * 512, (ch + 1) * 512)
                    wsl = slice(p * 128, (p + 1) * 128)

                    def a():
                        uid[0] += 1
                        st["ps"] = pproj.tile([128, 512], F32, tag="pj",
                                              name=f"pj{uid[0]}")
                        for k in range(nk // 2):
                            nc.tensor.matmul(st["ps"][:], w_sb[:, k, wsl],
                                             src[:, k, csl],
                                             start=(k == 0), stop=False)

                    def b():
                        for k in range(nk // 2, nk):
                            nc.tensor.matmul(st["ps"][:], w_sb[:, k, wsl],
                                             src[:, k, csl],
                                             start=False, stop=(k == nk - 1))
                        nc.vector.tensor_tensor(
                            dst[:, p, csl], st["ps"][:],
                            b_sb[:, p:p + 1].to_broadcast([128, 512]),
                            OP.add,
                        )

                    return a, b

                def make_v_unit(t):
                    def u():
                        uid[0] += 1
                        ps = pproj.tile([128, 512], F32, tag="pj",
                                        name=f"pv{uid[0]}")
                        for k in range(KT_C):
                            nc.tensor.matmul(
                                ps[:, 0:DSL],
                                ctxT_sb[:, k, t * 128:(t + 1) * 128],
                                wv_sb[:, k, :],
                                start=(k == 0), stop=(k == KT_C - 1),
                            )
                        nc.vector.tensor_tensor(
                            v2_sb[:, t, :, 0:DH],
                            ps[:, 0:DSL].rearrange("p (g d) -> p g d", d=DH),
                            bvr_sb.rearrange("p (g d) -> p g d", d=DH),
                            OP.add,
                        )
                    return u

                def make_o_unit(sc, st_, n, dma_eng=None):
                    def u():
                        uid[0] += 1
                        row = (sc * 4 + st_) * 128
                        ps = pproj.tile([128, 512], F32, tag="pj",
                                        name=f"po{uid[0]}")
                        for l in range(2):
                            nc.tensor.matmul(
                                ps[:],
                                avT_sb[:, l, row:row + 128],
                                wo_sb[:, l, n * 512:(n + 1) * 512],
                                start=(l == 0), stop=(l == 1),
                            )
                        os_sb = ospool.tile([128, 512], F16, tag="os")
                        nc.vector.tensor_copy(os_sb[:], ps[:])
                        (dma_eng or nc.sync).dma_start(
                            out[row:row + 128, n * 512:(n + 1) * 512],
                            os_sb[:],
                        )
                    return u

                # ---- static filler schedule: sched[(w, t)] = [units] ----
                windows = [(sc, p) for p in range(2) for sc in range(SCK)]
                sched = {}

                def put(w, t, u):
                    sched.setdefault((w, t), []).append(u)

                # kT p0 chunks 1..3 JIT inside window 0 (consumed at t=4c)
                for c in range(1, SCK):
                    a, b = make_qk_units(kT_sb, wk_sb, bk_sb, ctxT_sb,
                                         KT_C, 0, c)
                    put(0, (c - 1) * 4, a)
                    put(0, (c - 1) * 4 + 1, b)
                # v2 tiles 2..15 racing 2 ahead of window-0 consumption
                for t in range(2, TT):
                    put(0, t - 2, make_v_unit(t))
                # qT p0 chunk 1 late in window 0 (needed at window 1)
                a, b = make_qk_units(qT_sb, wq_sb, bq_sb, xT_sb, KT_E, 0, 1)
                put(0, 12, a)
                put(0, 14, b)
                # windows 1-2: kT p1 (4 chunks), qT p1 c0/c1, qT p0 c2/c3
                for c in range(2):
                    a, b = make_qk_units(kT_sb, wk_sb, bk_sb, ctxT_sb,
                                         KT_C, 1, c)
                    put(1, 8 * c + 0, a)
                    put(1, 8 * c + 2, b)
                a, b = make_qk_units(qT_sb, wq_sb, bq_sb, xT_sb, KT_E, 0, 2)
                put(1, 5, a)
                put(1, 7, b)
                a, b = make_qk_units(qT_sb, wq_sb, bq_sb, xT_sb, KT_E, 1, 0)
                put(1, 12, a)
                put(1, 14, b)
                for c in range(2, SCK):
                    a, b = make_qk_units(kT_sb, wk_sb, bk_sb, ctxT_sb,
                                         KT_C, 1, c)
                    put(2, 8 * (c - 2) + 0, a)
                    put(2, 8 * (c - 2) + 2, b)
                a, b = make_qk_units(qT_sb, wq_sb, bq_sb, xT_sb, KT_E, 1, 1)
                put(2, 5, a)
                put(2, 7, b)
                a, b = make_qk_units(qT_sb, wq_sb, bq_sb, xT_sb, KT_E, 0, 3)
                put(2, 12, a)
                put(2, 14, b)
                # window 3: qT p1 chunks 2,3
                a, b = make_qk_units(qT_sb, wq_sb, bq_sb, xT_sb, KT_E, 1, 2)
                put(3, 2, a)
                put(3, 4, b)
                a, b = make_qk_units(qT_sb, wq_sb, bq_sb, xT_sb, KT_E, 1, 3)
                put(3, 8, a)
                put(3, 10, b)
                # out-proj of chunk sc inside window (4 + sc + 1); its
                # avT inputs complete at the end of window (4 + sc)
                for sc in range(SCK - 1):
                    w = 5 + sc
                    i = 0
                    for st_ in range(4):
                        for n in range(2):
                            put(w, 2 * i, make_o_unit(sc, st_, n))
                            i += 1

                # ---- prelude: minimum inputs for window 0 ----
                a, b = make_qk_units(kT_sb, wk_sb, bk_sb, ctxT_sb, KT_C, 0, 0)
                a(); b()
                a, b = make_qk_units(qT_sb, wq_sb, bq_sb, xT_sb, KT_E, 0, 0)
                a(); b()
                make_v_unit(0)()
                make_v_unit(1)()

                # ---- attention windows ----
                for w, (sc, p) in enumerate(windows):
                    ssl = slice(sc * 512, (sc + 1) * 512)
                    avz = pavz.tile([128, 1024], F32, tag="avz",
                                    name=f"avz{w}")
                    scps, exs = {}, {}

                    def scores(t):
                        scp = psc.tile([128, 1024], F32, tag="sc",
                                       name=f"sc{w}_{t}")
                        for h in range(2):
                            hb = h * DH
                            nc.tensor.matmul(
                                scp[:, h * 512:(h + 1) * 512],
                                kT_sb[hb:hb + DH, p, t * 128:(t + 1) * 128],
                                qT_sb[hb:hb + DH, p, ssl],
                                start=True, stop=True,
                            )
                        scps[t] = scp

                    def expf(t):
                        ex = expool.tile([128, 1024], F16, tag="ex",
                                         name=f"ex{w}_{t}")
                        nc.scalar.activation(ex[:], scps.pop(t)[:], AF.Exp,
                                             scale=0.125)
                        exs[t] = ex

                    def av(t):
                        ex = exs.pop(t)
                        for h in range(2):
                            nc.tensor.matmul(
                                avz[:, h * 512:(h + 1) * 512],
                                v2_sb[:, t, p * 2 + h, :],
                                ex[:, h * 512:(h + 1) * 512],
                                start=(t == 0), stop=(t == TT - 1),
                            )

                    scores(0)
                    expf(0)
                    scores(1)
                    expf(1)
                    for t in range(TT):
                        for u in sched.pop((w, t), ()):
                            u()
                        av(t)
                        if t + 2 < TT:
                            scores(t + 2)
                            expf(t + 2)
                    # normalize via base-0 staged reciprocal_approx_fast.
                    # One full-tile copy frees the avz PSUM pair after ~1.2us
                    # (next window's first av matmul has a WAR dependency on
                    # the last avz reader), then everything reads SBUF.
                    rz = rzpool.tile([128, 1024], F32, tag="rz",
                                     name=f"rz{w}")
                    rzd = rzpool.tile([64, 2048], F32, tag="rzd",
                                      name=f"rzd{w}")
                    nc.vector.tensor_copy(rz[:], avz[:])
                    nc.vector.tensor_copy(rzd[0:DH, 0:1024], rz[DH:128, :])
                    for h in range(2):
                        hs = slice(h * 512, (h + 1) * 512)
                        nc.vector.reciprocal_approx_fast(
                            rzd[0:DH, 1024 + h * 512:1024 + (h + 1) * 512],
                            rzd[0:DH, hs])
                    for h in range(2):
                        hb = h * DH
                        nc.vector.tensor_tensor(
                            avT_sb[hb:hb + DH, p, ssl],
                            rz[0:DH, h * 512:(h + 1) * 512],
                            rzd[0:DH, 1024 + h * 512:1024 + (h + 1) * 512],
                            OP.mult,
                        )

                # ---- tail: last chunk's output projection (ACT engine is
                # idle by now — issue the output DMAs from its HWDGE queue
                # to skip the serial SP issue overhead) ----
                for st_ in range(4):
                    for n in range(2):
                        make_o_unit(SCK - 1, st_, n, dma_eng=nc.scalar)()
                assert not sched, f"unemitted fillers: {list(sched)}"

    nc.compile()
    return nc


def get_nc():
    if "nc" not in _NC_CACHE:
        _NC_CACHE["nc"] = _build_nc()
    return _NC_CACHE["nc"]


def make_in_maps(x, context, Wq, bq, Wk, bk, Wv, bv, Wo, bo):
    x = np.asarray(x, dtype=np.float32)
    context = np.asarray(context, dtype=np.float32)
    Wq = np.asarray(Wq, dtype=np.float32)
    Wk = np.asarray(Wk, dtype=np.float32)
    Wv = np.asarray(Wv, dtype=np.float32)
    Wo = np.asarray(Wo, dtype=np.float32)
    bq = np.asarray(bq, dtype=np.float32)
    bk = np.asarray(bk, dtype=np.float32)
    bv = np.asarray(bv, dtype=np.float32)

    xT = [np.ascontiguousarray(x[b].T).astype(np.float16) for b in range(B)]
    ctxT = [np.ascontiguousarray(context[b].T).astype(np.float16)
            for b in range(B)]
    in_maps = []
    for c in range(N_CORES):
        b, g = c // GROUPS, c % GROUPS
        sl = slice(g * DSL, (g + 1) * DSL)
        in_maps.append({
            "xT": xT[b],
            "ctxT": ctxT[b],
            "wq": Wq[:, sl].astype(np.float16),
            "wk": Wk[:, sl].astype(np.float16),
            "wv": Wv[:, sl].astype(np.float16),
            "wo": Wo[sl, :].astype(np.float16),
            "bq": np.ascontiguousarray(bq[sl].reshape(2, 128).T),
            "bk": np.ascontiguousarray(bk[sl].reshape(2, 128).T),
            "bvr": np.ascontiguousarray(
                np.tile(bv[sl].reshape(1, DSL), (128, 1))),
        })
    return in_maps


def run_sharded(inputs, trace=False):
    nc = get_nc()
    in_maps = make_in_maps(**inputs)
    res = bass_utils.run_bass_kernel_spmd(
        nc, in_maps, core_ids=list(range(N_CORES)), trace=trace,
    )
    bo = np.asarray(inputs["bo"], dtype=np.float32)
    full = np.empty((B, S, E), dtype=np.float32)
    for b in range(B):
        acc = res.results[b * GROUPS]["out"].astype(np.float32)
        for g in range(1, GROUPS):
            acc = acc + res.results[b * GROUPS + g]["out"].astype(np.float32)
        full[b] = acc + bo[None, :]
    return full, res.exec_time_ns


def kernel(**inputs) -> np.ndarray:
    return run_sharded(inputs)[0]


# revision 12
# speedup vs baseline: 1.0404x; 1.0404x over previous
"""Trainium2 Bass kernel for nn_CrossAttention (B=2, S=2048, E=1024, H=16, ctx=768).

Sharding: 4-way tensor-parallel over heads x 2-way data-parallel over batch.
Core c handles batch c//4 and heads 4*(c%4) .. 4*(c%4)+3.

Software-pipelined single-pass design:
  The attention inner loop is ACT(exp)-bound (~1.04 us per [128,1024] exp
  tile) while its PE work (scores pair + av pair) is only ~0.76 us, so all
  projection / output-projection matmuls are emitted as *fillers* inside
  the attention windows to run in the PE slack.  Windows are p-major
  ((sc,p)=(0..3,0) then (0..3,1)) so the p=1 projections have 3+ windows
  of slack to complete, and out-proj of chunk sc runs inside window
  (sc+1, 1).

Per-core dataflow (fp16 operands, fp32 PSUM):
  qT/kT = W-stationary projections producing [dh, S] layouts (bias added
          during the DVE PSUM->SBUF drain)
  v2    = ctxT-tile-stationary projection into [S-tile, head, v|ones]
          (bias added via a host-replicated [128,256] table on DVE)
  scT   = kT-tile x qT (K=64); head pair emitted on PE row groups 0/64
  exp   = ScalarE, fused 1/sqrt(dh) scale, PSUM -> SBUF fp16, one
          [128,1024] tile per (t, head-pair); 1-deep scores lookahead
  av/Z  = [v_h | ones] stationary: rows 0:64 unnormalized out.T, rows
          64:128 softmax denominator, pair packed in one [128,1024] PSUM
  norm  = single DVE divide per head (numerator rows / denominator rows)
  out   = avT-stationary x Wo, fp16 partial [S, E] per core

Host side: pre-transpose x/context, slice weights per head group, fp16
cast; sum the 4 per-batch fp16 partials + bo in fp32 on host.
"""
import numpy as np

import concourse.bass as bass
import concourse.mybir as mybir
import concourse.tile as tile
from concourse import bacc, bass_utils

F16 = mybir.dt.float16
F32 = mybir.dt.float32
AF = mybir.ActivationFunctionType
OP = mybir.AluOpType

B, S, E, C, H, DH = 2, 2048, 1024, 768, 16, 64
N_CORES = 8
GROUPS = 4            # head groups (tensor parallel)
HPG = H // GROUPS     # heads per group = 4
DSL = HPG * DH        # feature slice per core = 256
KT_E = E // 128       # 8 k-tiles for x projections
KT_C = C // 128       # 6 k-tiles for context projections
SCK = S // 512        # 4 s-chunks
TT = S // 128         # 16 t-tiles

_NC_CACHE = {}


def _build_nc():
    nc = bacc.Bacc("TRN2", target_bir_lowering=False, debug=False,
                   num_devices=N_CORES)

    xT = nc.dram_tensor("xT", [E, S], F16, kind="ExternalInput").ap()
    ctxT = nc.dram_tensor("ctxT", [C, S], F16, kind="ExternalInput").ap()
    wq = nc.dram_tensor("wq", [E, DSL], F16, kind="ExternalInput").ap()
    wk = nc.dram_tensor("wk", [C, DSL], F16, kind="ExternalInput").ap()
    wv = nc.dram_tensor("wv", [C, DSL], F16, kind="ExternalInput").ap()
    wo = nc.dram_tensor("wo", [DSL, E], F16, kind="ExternalInput").ap()
    bq = nc.dram_tensor("bq", [128, 2], F32, kind="ExternalInput").ap()
    bk = nc.dram_tensor("bk", [128, 2], F32, kind="ExternalInput").ap()
    bvr = nc.dram_tensor("bvr", [128, DSL], F32, kind="ExternalInput").ap()
    out = nc.dram_tensor("out", [S, E], F16, kind="ExternalOutput").ap()

    xT_r = xT.rearrange("(o p) s -> p o s", p=128)
    ctxT_r = ctxT.rearrange("(o p) s -> p o s", p=128)
    wq_r = wq.rearrange("(o p) m -> p o m", p=128)
    wk_r = wk.rearrange("(o p) m -> p o m", p=128)

    with tile.TileContext(nc) as tc:
        with (
            tc.tile_pool(name="const", bufs=1) as cpool,
            tc.tile_pool(name="ex", bufs=4) as expool,
            tc.tile_pool(name="os", bufs=4) as ospool,
            tc.tile_pool(name="rz", bufs=1) as rzpool,
        ):
            wq_sb = cpool.tile([128, KT_E, DSL], F16)
            wk_sb = cpool.tile([128, KT_C, DSL], F16)
            wv_sb = cpool.tile([128, KT_C, DSL], F16)
            wo_sb = cpool.tile([128, 2, E], F16)
            bq_sb = cpool.tile([128, 2], F32)
            bk_sb = cpool.tile([128, 2], F32)
            bvr_sb = cpool.tile([128, DSL], F32)
            warm_sb = cpool.tile([1, 8], F32)
            ctxT_sb = cpool.tile([128, KT_C, S], F16)
            xT_sb = cpool.tile([128, KT_E, S], F16)

            qT_sb = cpool.tile([128, 2, S], F16)
            kT_sb = cpool.tile([128, 2, S], F16)
            # per (t, head): 128 cols = [v_h (64) | ones (64)] so one matmul
            # yields av rows 0:64 and the replicated softmax denominator
            # rows 64:128 in a single PSUM bank
            v2_sb = cpool.tile([128, TT, HPG, 128], F16)
            avT_sb = cpool.tile([128, 2, S], F16)

            # ---- DMAs in need-order, few large issues: each dma_start
            # costs ~625ns of serial SP-sequencer issue time, while one
            # issue's descriptors spread across all 16 DMA queues ----
            nc.sync.dma_start(wk_sb[:], wk_r[:])
            nc.sync.dma_start(ctxT_sb[:, 0:3, 0:512], ctxT_r[:, 0:3, 0:512])
            nc.sync.dma_start(ctxT_sb[:, 3:6, 0:512], ctxT_r[:, 3:6, 0:512])
            nc.sync.dma_start(wq_sb[:], wq_r[:])
            nc.sync.dma_start(xT_sb[:, 0:4, 0:512], xT_r[:, 0:4, 0:512])
            nc.sync.dma_start(xT_sb[:, 4:8, 0:512], xT_r[:, 4:8, 0:512])
            nc.sync.dma_start(bk_sb[:], bk[:])
            nc.sync.dma_start(bq_sb[:], bq[:])
            nc.sync.dma_start(wv_sb[:], wv.rearrange("(o p) m -> p o m", p=128))
            nc.sync.dma_start(bvr_sb[:], bvr[:])
            # per-chunk issues: JIT fillers then wait only on the chunk
            # they consume, not the whole remaining transfer
            for ch in range(1, SCK):
                csl = slice(ch * 512, (ch + 1) * 512)
                nc.sync.dma_start(ctxT_sb[:, :, csl], ctxT_r[:, :, csl])
            for ch in range(1, SCK):
                csl = slice(ch * 512, (ch + 1) * 512)
                nc.sync.dma_start(xT_sb[:, :, csl], xT_r[:, :, csl])
            nc.sync.dma_start(wo_sb[:], wo.rearrange("(l p) n -> p l n", p=128))

            # ones halves of v2 (written once; v halves overwritten by proj)
            nc.vector.memset(v2_sb[:, :, :, DH:128], 1.0)
            wsrc_sb = cpool.tile([128, 512], F16)
            nc.vector.memset(wsrc_sb[:], 0.0)
            nc.vector.memset(warm_sb[:], 0.0)
            # pull the exp table load off the critical path
            nc.scalar.activation(warm_sb[:], warm_sb[:], AF.Exp)

            with (
                tc.tile_pool(name="psc", bufs=2, space="PSUM") as psc,
                tc.tile_pool(name="pproj", bufs=2, space="PSUM") as pproj,
                tc.tile_pool(name="pavz", bufs=1, space="PSUM") as pavz,
            ):
                uid = [0]

                def qk_unit(dst, w_sb, b_sb, src, nk, p, ch, half):
                    """Half of one 512-col projection chunk (contract split).

                    half=0 allocates the psum + first nk//2 matmuls;
                    half=1 finishes the contraction + DVE bias drain.
                    Returns state via closure dict."""
                    pass  # replaced below

                # projection chunk as two emission units sharing one psum
                def make_qk_units(dst, w_sb, b_sb, src, nk, p, ch):
                    st = {}
                    csl = slice(ch * 512, (ch + 1) * 512)
                    wsl = slice(p * 128, (p + 1) * 128)

                    def a():
                        uid[0] += 1
                        st["ps"] = pproj.tile([128, 512], F32, tag="pj",
                                              name=f"pj{uid[0]}")
                        for k in range(nk // 2):
                            nc.tensor.matmul(st["ps"][:], w_sb[:, k, wsl],
                                             src[:, k, csl],
                                             start=(k == 0), stop=False)

                    def b():
                        for k in range(nk // 2, nk):
                            nc.tensor.matmul(st["ps"][:], w_sb[:, k, wsl],
                                             src[:, k, csl],
                                             start=False, stop=(k == nk - 1))
                        nc.vector.tensor_tensor(
                            dst[:, p, csl], st["ps"][:],
                            b_sb[:, p:p + 1].to_broadcast([128, 512]),
                            OP.add,
                        )

                    return a, b

                def make_v_unit(t):
                    def u():
                        uid[0] += 1
                        ps = pproj.tile([128, 512], F32, tag="pj",
                                        name=f"pv{uid[0]}")
                        for k in range(KT_C):
                            nc.tensor.matmul(
                                ps[:, 0:DSL],
                                ctxT_sb[:, k, t * 128:(t + 1) * 128],
                                wv_sb[:, k, :],
                                start=(k == 0), stop=(k == KT_C - 1),
                            )
                        nc.vector.tensor_tensor(
                            v2_sb[:, t, :, 0:DH],
                            ps[:, 0:DSL].rearrange("p (g d) -> p g d", d=DH),
                            bvr_sb.rearrange("p (g d) -> p g d", d=DH),
                            OP.add,
                        )
                    return u

                def make_o_unit(sc, st_, n, dma_eng=None):
                    def u():
                        uid[0] += 1
                        row = (sc * 4 + st_) * 128
                        ps = pproj.tile([128, 512], F32, tag="pj",
                                        name=f"po{uid[0]}")
                        for l in range(2):
                            nc.tensor.matmul(
                                ps[:],
                                avT_sb[:, l, row:row + 128],
                                wo_sb[:, l, n * 512:(n + 1) * 512],
                                start=(l == 0), stop=(l == 1),
                            )
                        os_sb = ospool.tile([128, 512], F16, tag="os")
                        nc.vector.tensor_copy(os_sb[:], ps[:])
                        (dma_eng or nc.sync).dma_start(
                            out[row:row + 128, n * 512:(n + 1) * 512],
                            os_sb[:],
                        )
                    return u

                # ---- static filler schedule: sched[(w, t)] = [units] ----
                windows = [(sc, p) for p in range(2) for sc in range(SCK)]
                sched = {}

                def put(w, t, u):
                    sched.setdefault((w, t), []).append(u)

                # kT p0 chunks 1..3 JIT inside window 0 (consumed at t=4c)
                for c in range(1, SCK):
                    a, b = make_qk_units(kT_sb, wk_sb, bk_sb, ctxT_sb,
                                         KT_C, 0, c)
                    put(0, (c - 1) * 4, a)
                    put(0, (c - 1) * 4 + 1, b)
                # v2 tiles 2..15 racing 2 ahead of window-0 consumption
                for t in range(2, TT):
                    put(0, t - 2, make_v_unit(t))
                # qT p0 chunk 1 late in window 0 (needed at window 1)
                a, b = make_qk_units(qT_sb, wq_sb, bq_sb, xT_sb, KT_E, 0, 1)
                put(0, 12, a)
                put(0, 14, b)
                # windows 1-2: kT p1 (4 chunks), qT p1 c0/c1, qT p0 c2/c3
                for c in range(2):
                    a, b = make_qk_units(kT_sb, wk_sb, bk_sb, ctxT_sb,
                                         KT_C, 1, c)
                    put(1, 8 * c + 0, a)
                    put(1, 8 * c + 2, b)
                a, b = make_qk_units(qT_sb, wq_sb, bq_sb, xT_sb, KT_E, 0, 2)
                put(1, 5, a)
                put(1, 7, b)
                a, b = make_qk_units(qT_sb, wq_sb, bq_sb, xT_sb, KT_E, 1, 0)
                put(1, 12, a)
                put(1, 14, b)
                for c in range(2, SCK):
                    a, b = make_qk_units(kT_sb, wk_sb, bk_sb, ctxT_sb,
                                         KT_C, 1, c)
                    put(2, 8 * (c - 2) + 0, a)
                    put(2, 8 * (c - 2) + 2, b)
                a, b = make_qk_units(qT_sb, wq_sb, bq_sb, xT_sb, KT_E, 1, 1)
                put(2, 5, a)
                put(2, 7, b)
                a, b = make_qk_units(qT_sb, wq_sb, bq_sb, xT_sb, KT_E, 0, 3)
                put(2, 12, a)
                put(2, 14, b)
                # window 3: qT p1 chunks 2,3
                a, b = make_qk_units(qT_sb, wq_sb, bq_sb, xT_sb, KT_E, 1, 2)
                put(3, 2, a)
                put(3, 4, b)
                a, b = make_qk_units(qT_sb, wq_sb, bq_sb, xT_sb, KT_E, 1, 3)
                put(3, 8, a)
                put(3, 10, b)
                # out-proj of chunk sc inside window (4 + sc + 1); its
                # avT inputs complete at the end of window (4 + sc)
                for sc in range(SCK - 1):
                    w = 5 + sc
                    i = 0
                    for st_ in range(4):
                        for n in range(2):
                            put(w, 2 * i, make_o_unit(sc, st_, n))
                            i += 1

                # ---- PE p-state warm-up: dependency-free matmuls run
                # during the initial DMA wait so the 3us continuous-busy
                # ramp to full clock completes before the real prelude ----
                wps = pproj.tile([128, 512], F32, tag="pj", name="warmps")
                for i in range(12):
                    nc.tensor.matmul(wps[:], wsrc_sb[:, 0:128],
                                     wsrc_sb[:, 0:512],
                                     start=(i == 0), stop=(i == 11))

                # ---- prelude: minimum inputs for window 0 ----
                a, b = make_qk_units(kT_sb, wk_sb, bk_sb, ctxT_sb, KT_C, 0, 0)
                a(); b()
                a, b = make_qk_units(qT_sb, wq_sb, bq_sb, xT_sb, KT_E, 0, 0)
                a(); b()
                make_v_unit(0)()
                make_v_unit(1)()

                # ---- attention windows ----
                for w, (sc, p) in enumerate(windows):
                    ssl = slice(sc * 512, (sc + 1) * 512)
                    avz = pavz.tile([128, 1024], F32, tag="avz",
                                    name=f"avz{w}")
                    scps, exs = {}, {}

                    def scores(t):
                        scp = psc.tile([128, 1024], F32, tag="sc",
                                       name=f"sc{w}_{t}")
                        for h in range(2):
                            hb = h * DH
                            nc.tensor.matmul(
                                scp[:, h * 512:(h + 1) * 512],
                                kT_sb[hb:hb + DH, p, t * 128:(t + 1) * 128],
                                qT_sb[hb:hb + DH, p, ssl],
                                start=True, stop=True,
                            )
                        scps[t] = scp

                    def expf(t):
                        ex = expool.tile([128, 1024], F16, tag="ex",
                                         name=f"ex{w}_{t}")
                        nc.scalar.activation(ex[:], scps.pop(t)[:], AF.Exp,
                                             scale=0.125)
                        exs[t] = ex

                    def av(t):
                        ex = exs.pop(t)
                        for h in range(2):
                            nc.tensor.matmul(
                                avz[:, h * 512:(h + 1) * 512],
                                v2_sb[:, t, p * 2 + h, :],
                                ex[:, h * 512:(h + 1) * 512],
                                start=(t == 0), stop=(t == TT - 1),
                            )

                    scores(0)
                    expf(0)
                    scores(1)
                    expf(1)
                    for t in range(TT):
                        for u in sched.pop((w, t), ()):
                            u()
                        av(t)
                        if t + 2 < TT:
                            scores(t + 2)
                            expf(t + 2)
                    # normalize via base-0 staged reciprocal_approx_fast.
                    # One full-tile copy frees the avz PSUM pair after ~1.2us
                    # (next window's first av matmul has a WAR dependency on
                    # the last avz reader), then everything reads SBUF.
                    rz = rzpool.tile([128, 1024], F32, tag="rz",
                                     name=f"rz{w}")
                    rzd = rzpool.tile([64, 2048], F32, tag="rzd",
                                      name=f"rzd{w}")
                    nc.vector.tensor_copy(rz[:], avz[:])
                    nc.vector.tensor_copy(rzd[0:DH, 0:1024], rz[DH:128, :])
                    for h in range(2):
                        hs = slice(h * 512, (h + 1) * 512)
                        nc.vector.reciprocal_approx_fast(
                            rzd[0:DH, 1024 + h * 512:1024 + (h + 1) * 512],
                            rzd[0:DH, hs])
                    for h in range(2):
                        hb = h * DH
                        nc.vector.tensor_tensor(
                            avT_sb[hb:hb + DH, p, ssl],
                            rz[0:DH, h * 512:(h + 1) * 512],
                            rzd[0:DH, 1024 + h * 512:1024 + (h + 1) * 512],
                            OP.mult,
                        )

                # ---- tail: last chunk's output projection (ACT engine is
                # idle by now — issue the output DMAs from its HWDGE queue
                # to skip the serial SP issue overhead) ----
                for st_ in range(4):
                    for n in range(2):
                        make_o_unit(SCK - 1, st_, n, dma_eng=nc.scalar)()
                assert not sched, f"unemitted fillers: {list(sched)}"

    nc.compile()
    return nc


def get_nc():
    if "nc" not in _NC_CACHE:
        _NC_CACHE["nc"] = _build_nc()
    return _NC_CACHE["nc"]


def make_in_maps(x, context, Wq, bq, Wk, bk, Wv, bv, Wo, bo):
    x = np.asarray(x, dtype=np.float32)
    context = np.asarray(context, dtype=np.float32)
    Wq = np.asarray(Wq, dtype=np.float32)
    Wk = np.asarray(Wk, dtype=np.float32)
    Wv = np.asarray(Wv, dtype=np.float32)
    Wo = np.asarray(Wo, dtype=np.float32)
    bq = np.asarray(bq, dtype=np.float32)
    bk = np.asarray(bk, dtype=np.float32)
    bv = np.asarray(bv, dtype=np.float32)

    xT = [np.ascontiguousarray(x[b].T).astype(np.float16) for b in range(B)]
    ctxT = [np.ascontiguousarray(context[b].T).astype(np.float16)
            for b in range(B)]
    in_maps = []
    for c in range(N_CORES):
        b, g = c // GROUPS, c % GROUPS
        sl = slice(g * DSL, (g + 1) * DSL)
        in_maps.append({
            "xT": xT[b],
            "ctxT": ctxT[b],
            "wq": Wq[:, sl].astype(np.float16),
            "wk": Wk[:, sl].astype(np.float16),
            "wv": Wv[:, sl].astype(np.float16),
            "wo": Wo[sl, :].astype(np.float16),
            "bq": np.ascontiguousarray(bq[sl].reshape(2, 128).T),
            "bk": np.ascontiguousarray(bk[sl].reshape(2, 128).T),
            "bvr": np.ascontiguousarray(
                np.tile(bv[sl].reshape(1, DSL), (128, 1))),
        })
    return in_maps


def run_sharded(inputs, trace=False):
    nc = get_nc()
    in_maps = make_in_maps(**inputs)
    res = bass_utils.run_bass_kernel_spmd(
        nc, in_maps, core_ids=list(range(N_CORES)), trace=trace,
    )
    bo = np.asarray(inputs["bo"], dtype=np.float32)
    full = np.empty((B, S, E), dtype=np.float32)
    for b in range(B):
        acc = res.results[b * GROUPS]["out"].astype(np.float32)
        for g in range(1, GROUPS):
            acc = acc + res.results[b * GROUPS + g]["out"].astype(np.float32)
        full[b] = acc + bo[None, :]
    return full, res.exec_time_ns


def kernel(**inputs) -> np.ndarray:
    return run_sharded(inputs)[0]
